# revision 17
# baseline (speedup 1.0000x reference)
"""CaLCS (soft-LCS) loss kernel for Trainium2, 8 NeuronCores, data-parallel over batch.

Problem (hardcoded shapes): batch [8, 512, 32000] f32 logits, docs [8, 512] int64.
  probs = softmax(batch, axis=2); p[b,i,j] = probs[b, i, docs[b,j]]
  D[i,j] = p*(1+D[i-1,j-1]) + (1-p)*max(D[i-1,j], D[i,j-1])
  loss = -log(mean_b min(D[511,511], 100) / 512)

Sharding: one batch element per core (SPMD, same program).

This target executes ~one instruction per ~40us regardless of operand size,
so the design minimizes instruction count:

Phase 1 (per core, ~75 instructions): stream the [512, 32000] logits in
  8 big chunks, exp + row-sum on ACT (randn logits: exp is fp32-safe without
  max subtraction); normalize the host-pre-sliced logit columns into
  p = exp(x)/Z, q = 1-p; compute R = prefix-prod(q) (one scan per row
  group), invR = 1/R, pinv = p*invR; pack per-row vectors
  [pinv_i, invRshift_i, R_i] into DRAM.

Phase 2 (~4 instructions per DP row): the row recurrence
    a_j = K_j + q_j * max(b_j, a_{j-1}),   K_j = p_j * (1 + b_{j-1})
  (a = D row i, b = D row i-1) normalized by alpha_j = a_j / R_j becomes a
  pure (max,+) scan:
    alpha_j = max(v_j, alpha_{j-1}) + Khat_j,
    v_j = b_j / R_{j-1},  Khat_j = (b_{j-1} + 1) * p_j / R_j
  which is exactly one hardware tensor_tensor_scan (op0=max, op1=add).
  Per row: Khat (scalar_tensor_tensor), v (tensor_tensor), the scan, and
  a = alpha * R (tensor_tensor), all [1, 512] on partition 0, plus one
  staging DMA per 8 rows for the packed per-row constants.

Host: gathers the 8 clamped D values, returns -log(mean/512).
"""

import numpy as np

import bass_rust
import concourse.bass as bass
import concourse.tile as tile
import concourse.mybir as mybir
from concourse import bass_utils

# ---- problem constants (hardcoded per contract) ----
B = 8
R = 512          # generation steps (rows of DP grid)
V = 32000        # vocab
C = 512          # doc length (cols of DP grid)
CLAMP = 100.0
P = 128          # SBUF partitions
NGRP = R // P    # 4 row groups
VCHUNK = 16000
NCHUNK = V // VCHUNK   # 2 chunks per row group
FW = C           # packed field width
# packed row == staged row: [F(512), E(512), d1(1024: khatp/pinv),
# d0(1024: v/-BIG)] so block staging is one contiguous DMA on one queue
ROWSTRIDE = 6 * FW
SLOTSTRIDE = 6 * FW
BLK = 11         # rows staged per DMA (single slot)
NEGBIG = -1.0e30

F32 = mybir.dt.float32
ALU = mybir.AluOpType
ACTF = mybir.ActivationFunctionType


def _patched_drain_and_barrier(self, tick_clock, wait_clock):
    """Split the kernel-tail drain's sem waits across multiple drain
    instructions — core_v3 codegen rejects multi-wait CTRL instructions."""
    from concourse.tile import ScopedClock

    nc = self.nc
    probe = nc.sync.drain()
    wait_clock.add_sem_waits(probe.ins, ScopedClock({None: tick_clock.global_clock}))
    waits = list(probe.ins.sync_info.on_wait) if probe.ins.sync_info else []
    if len(waits) > 1:
        probe.ins.sync_info = bass_rust.SyncInfo(on_wait=waits[:1], on_update=[])
        for i in range(1, len(waits)):
            d = nc.sync.drain()
            d.ins.sync_info = bass_rust.SyncInfo(on_wait=[waits[i]], on_update=[])
    nc.all_engine_barrier()
    popped = nc._tile_sem_poison_stack.pop()
    assert popped is self._sem_poison
    nc.clear_and_free_semaphores(list(self.sems.allocated().values()))
    nc.all_engine_barrier()


tile.TileContext._drain_and_barrier = _patched_drain_and_barrier


def _split_multi_waits(nc: bass.Bass):
    """Walrus codegen for TRN2 accepts at most one sem wait per instruction.
    Hoist extra waits into same-engine NoOp/Drain instructions inserted
    immediately before the offending instruction."""
    n_split = 0
    for fn in nc.m.functions:
        for blk in fn.blocks:
            il = blk.instructions
            i = 0
            while i < len(il):
                inst = il[i]
                si = inst.sync_info
                if si is not None and len(si.on_wait) > 1:
                    waits = list(si.on_wait)
                    inst.sync_info = bass_rust.SyncInfo(
                        on_wait=[waits[0]], on_update=list(si.on_update)
                    )
                    for k, w in enumerate(waits[1:]):
                        if inst.engine == mybir.EngineType.PE:
                            filler = mybir.InstDrain(
                                name=f"wsplit-{inst.name}-{k}", engine=inst.engine,
                                sync_info=bass_rust.SyncInfo(on_wait=[w], on_update=[]),
                            )
                        else:
                            filler = mybir.InstNoOp(
                                name=f"wsplit-{inst.name}-{k}", engine=inst.engine,
                                sync_info=bass_rust.SyncInfo(on_wait=[w], on_update=[]),
                            )
                        il.insert(i, filler)
                        i += 1
                        n_split += 1
                i += 1
    return n_split


def build_nc(timing_reps: int = 0, *, dp_rows: int = R,
             do_phase1: bool = True, do_dp: bool = True) -> bass.Bass:
    """timing_reps=0: normal build (external inputs). timing_reps=K>0:
    inputs are Internal DRAM (zero-filled on device) and the whole body is
    repeated K times with barriers between reps, so wall-clock differences
    between rep counts isolate per-invocation device time."""
    nc = bass.Bass(trn_type="TRN2")
    kind = "Internal" if timing_reps else "ExternalInput"
    x = nc.dram_tensor("x", [R, V], F32, kind=kind)
    cols = nc.dram_tensor("cols", [R, C], F32, kind=kind)
    out = nc.dram_tensor("out", [1, 1], F32, kind="ExternalOutput")
    packed = nc.dram_tensor("packed", [R * ROWSTRIDE], F32, kind="Internal")
    rtmp = nc.dram_tensor("rtmp", [R * FW], F32, kind="Internal")

    with tile.TileContext(nc) as tc:
        with tc.tile_pool(name="keep", bufs=1) as keep:
            if timing_reps:
                with tc.tile_pool(name="zpool", bufs=1) as zpool:
                    zx = zpool.tile([P, VCHUNK], F32, tag="zx")
                    nc.vector.memset(zx[:, :], 0.0)
                    for grp in range(NGRP):
                        for k in range(NCHUNK):
                            nc.gpsimd.dma_start(
                                out=x[grp * P:(grp + 1) * P,
                                      k * VCHUNK:(k + 1) * VCHUNK],
                                in_=zx[:, :])
                        nc.gpsimd.dma_start(
                            out=cols[grp * P:(grp + 1) * P, :], in_=zx[:, :C])
                tc.strict_bb_all_engine_barrier()

            def emit_body():
                # ---------- phase 1 ----------
                if do_phase1:
                    with (
                        tc.tile_pool(name="chunks", bufs=2) as chunks,
                        tc.tile_pool(name="p1", bufs=1) as p1,
                    ):
                        ones = p1.tile([P, C], F32, tag="ones")
                        nc.gpsimd.memset(ones[:, :], 1.0)
                        # pre-fill d0 odd slots (-BIG) for all rows in DRAM
                        nbt = p1.tile([P, R * FW // P], F32, tag="nbt")
                        nc.gpsimd.memset(nbt[:, :], NEGBIG)
                        nc.sync.dma_start(
                            out=bass.AP(tensor=packed[:].tensor,
                                        offset=4 * FW + 1,
                                        ap=[[ROWSTRIDE, R], [2, FW]]),
                            in_=nbt[:, :])
                        for grp in range(NGRP):
                            r0 = grp * P
                            sums = p1.tile([P, NCHUNK], F32, tag="sums",
                                           name="sums")
                            for k in range(NCHUNK):
                                t = chunks.tile([P, VCHUNK], F32, tag="stream",
                                                name="stream_t")
                                nc.sync.dma_start(
                                    out=t[:, :],
                                    in_=x[r0:r0 + P,
                                          k * VCHUNK:(k + 1) * VCHUNK],
                                )
                                nc.scalar.activation(
                                    out=t[:, :], in_=t[:, :], func=ACTF.Exp,
                                    accum_out=sums[:, k:k + 1],
                                )
                            z = p1.tile([P, 1], F32, tag="z", name="zz")
                            nc.vector.tensor_reduce(
                                out=z[:, :], in_=sums[:, :],
                                axis=mybir.AxisListType.X, op=ALU.add,
                            )
                            rcp = p1.tile([P, 1], F32, tag="rcp", name="rcp")
                            nc.vector.reciprocal(out=rcp[:, :], in_=z[:, :])
                            nrcp = p1.tile([P, 1], F32, tag="nrcp", name="nrcp")
                            nc.vector.tensor_scalar(
                                out=nrcp[:, :], in0=rcp[:, :], scalar1=-1.0,
                                scalar2=None, op0=ALU.mult,
                            )
                            ct = p1.tile([P, C], F32, tag="ct", name="ct")
                            nc.sync.dma_start(out=ct[:, :],
                                              in_=cols[r0:r0 + P, :])
                            nc.scalar.activation(out=ct[:, :], in_=ct[:, :],
                                                 func=ACTF.Exp)
                            qt = p1.tile([P, C], F32, tag="qt", name="qt")
                            # q = 1 - exp(x)/Z
                            nc.vector.tensor_scalar(
                                out=qt[:, :], in0=ct[:, :], scalar1=nrcp[:, :],
                                scalar2=1.0, op0=ALU.mult, op1=ALU.add,
                            )
                            # p = exp(x)/Z  (in place)
                            nc.vector.tensor_scalar(
                                out=ct[:, :], in0=ct[:, :], scalar1=rcp[:, :],
                                scalar2=None, op0=ALU.mult,
                            )
                            # Rext[:, 0] = 1; Rext[:, 1+j] = prod_{t<=j} q_t
                            rext = p1.tile([P, C + 1], F32, tag="rext",
                                           name="rext")
                            nc.gpsimd.memset(rext[:, 0:1], 1.0)
                            nc.vector.tensor_tensor_scan(
                                out=rext[:, 1:C + 1], data0=qt[:, :],
                                data1=ones[:, :], initial=1.0,
                                op0=ALU.mult, op1=ALU.mult,
                            )
                            irext = p1.tile([P, C + 1], F32, tag="irext",
                                            name="irext")
                            nc.vector.reciprocal(out=irext[:, :],
                                                 in_=rext[:, :])
                            # pinv = p * invR   (in place over ct)
                            nc.gpsimd.tensor_tensor(
                                out=ct[:, :], in0=ct[:, :],
                                in1=irext[:, 1:C + 1], op=ALU.mult,
                            )
                            # R rows to rtmp (for prev-row access in pass B)
                            nc.sync.dma_start(
                                out=bass.AP(tensor=rtmp[:].tensor,
                                            offset=r0 * FW,
                                            ap=[[FW, P], [1, FW]]),
                                in_=rext[:, 1:C + 1])
                            # Rp[ph, 0:513] = [1, R rows of i-1] where
                            # i-1 = r0 + ph - 1 (ones when i == 0)
                            rp = p1.tile([P, C + 1], F32, tag="rp", name="rp")
                            nc.gpsimd.memset(rp[:, :], 1.0)
                            if grp == 0:
                                nc.sync.dma_start(
                                    out=rp[1:P, 1:C + 1],
                                    in_=bass.AP(tensor=rtmp[:].tensor,
                                                offset=0,
                                                ap=[[FW, P - 1], [1, FW]]))
                            else:
                                nc.sync.dma_start(
                                    out=rp[:, 1:C + 1],
                                    in_=bass.AP(tensor=rtmp[:].tensor,
                                                offset=(r0 - 1) * FW,
                                                ap=[[FW, P], [1, FW]]))
                            # F_j = R^(i-1)_{j-1} * pinv_j ; E_j = R^(i-1)_j / R^(i)_{j-1}
                            fe2 = p1.tile([P, 2 * C], F32, tag="fe2",
                                          name="fe2")
                            nc.gpsimd.tensor_tensor(
                                out=fe2[:, 0:C], in0=rp[:, 0:C], in1=ct[:, :],
                                op=ALU.mult)
                            nc.gpsimd.tensor_tensor(
                                out=fe2[:, C:2 * C], in0=rp[:, 1:C + 1],
                                in1=irext[:, 0:C], op=ALU.mult)
                            # pack per row i = 128*grp + ph:
                            #   [0,512)     F
                            #   [512,1024)  E
                            #   [1024,2048) d1 template: odd slots = pinv
                            nc.sync.dma_start(
                                out=bass.AP(tensor=packed[:].tensor,
                                            offset=r0 * ROWSTRIDE,
                                            ap=[[ROWSTRIDE, P], [1, 2 * FW]]),
                                in_=fe2[:, :])
                            nc.sync.dma_start(
                                out=bass.AP(tensor=packed[:].tensor,
                                            offset=r0 * ROWSTRIDE + 2 * FW + 1,
                                            ap=[[ROWSTRIDE, P], [2, FW]]),
                                in_=ct[:, :])

                # ---------- phase 2: row scans on partition 0 ----------
                if do_dp:
                    with tc.tile_pool(name="dp", bufs=1) as dp:
                        # alpha buffers [1, 2C+1]: col 0 = alpha_{-1} guard
                        # (0); scan writes cols 1..2C; alpha_j at col 2j+2.
                        a0 = dp.tile([1, 2 * C + 1], F32, tag="a0")
                        a1 = dp.tile([1, 2 * C + 1], F32, tag="a1")
                        nc.gpsimd.memset(a0[:, :], 0.0)
                        nc.gpsimd.memset(a1[:, :], 0.0)
                        abufs = [a0, a1]
                        # two staging slots, each BLK rows of
                        # [F, E, d1(khatp/pinv), d0(v/-BIG)]; d0 odd slots
                        # are -BIG constants persisted across slot reuse.
                        slots = [dp.tile([1, BLK * SLOTSTRIDE], F32,
                                         tag="slot0", name="slot0")]
                        for i in range(dp_rows):
                            if i % BLK == 0:
                                sl = slots[0]
                                nblk = min(BLK, dp_rows - i)
                                dst = bass.AP(
                                    tensor=sl.tensor, offset=sl.offset,
                                    ap=[sl.ap[0], [1, nblk * ROWSTRIDE]],
                                )
                                src = bass.AP(
                                    tensor=packed[:].tensor,
                                    offset=i * ROWSTRIDE,
                                    ap=[[1, nblk * ROWSTRIDE]],
                                )
                                nc.sync.dma_start(out=dst, in_=src)
                            so = sl.offset + (i % BLK) * SLOTSTRIDE
                            aprev = abufs[i % 2]
                            acur = abufs[(i + 1) % 2]
                            # vk: block0 khatp_j = alpha_{j-1}*F_j -> d1 even
                            #     block1 v_j    = alpha_j    *E_j -> d0 even
                            ain = bass.AP(
                                tensor=aprev.tensor, offset=aprev.offset,
                                ap=[aprev.ap[0], [2, 2], [2, C]],
                            )
                            fe = bass.AP(
                                tensor=sl.tensor, offset=so,
                                ap=[sl.ap[0], [FW, 2], [1, C]],
                            )
                            vkout = bass.AP(
                                tensor=sl.tensor, offset=so + 2 * FW,
                                ap=[sl.ap[0], [2 * FW, 2], [2, C]],
                            )
                            nc.vector.tensor_tensor(
                                out=vkout, in0=ain, in1=fe, op=ALU.mult,
                            )
                            # interleaved scan over 2C elements:
                            #   even step: s = max(v_j, s) + khatp_j
                            #   odd step:  s = max(-BIG, s) + pinv_j
                            d1 = bass.AP(tensor=sl.tensor, offset=so + 2 * FW,
                                         ap=[sl.ap[0], [1, 2 * C]])
                            d0 = bass.AP(tensor=sl.tensor, offset=so + 4 * FW,
                                         ap=[sl.ap[0], [1, 2 * C]])
                            nc.vector.tensor_tensor_scan(
                                out=acur[0:1, 1:2 * C + 1], data0=d0,
                                data1=d1, initial=0.0,
                                op0=ALU.max, op1=ALU.add,
                            )
                        # epilogue: a_511 = alpha_511 * R_511 (R from rtmp)
                        rlast = dp.tile([1, 1], F32, tag="rlast")
                        nc.sync.dma_start(
                            out=rlast[:, :],
                            in_=bass.AP(tensor=rtmp[:].tensor,
                                        offset=(dp_rows - 1) * FW + C - 1,
                                        ap=[[1, 1]]))
                        final = abufs[dp_rows % 2]
                        dres = dp.tile([1, 1], F32, tag="dres")
                        nc.gpsimd.tensor_tensor(
                            out=dres[:, :], in0=final[0:1, 2 * C:2 * C + 1],
                            in1=rlast[:, :], op=ALU.mult)
                        nc.sync.dma_start(out=out[:, :], in_=dres[:, :])
            for _rep in range(max(1, timing_reps)):
                if _rep:
                    tc.strict_bb_all_engine_barrier()
                emit_body()

    _split_multi_waits(nc)
    return nc


def kernel(batch: np.ndarray, docs: np.ndarray) -> np.ndarray:
    batch = np.ascontiguousarray(np.asarray(batch, dtype=np.float32))
    docs = np.asarray(docs)
    assert batch.shape == (B, R, V) and docs.shape == (B, C)

    nc = build_nc()
    in_maps = []
    for b in range(B):
        cols_b = np.ascontiguousarray(batch[b][:, docs[b].astype(np.int64)])
        in_maps.append({"x": batch[b], "cols": cols_b})

    res = bass_utils.run_bass_kernel_spmd(nc, in_maps, core_ids=list(range(B)))
    d_vals = np.array(
        [res.results[b]["out"][0, 0] for b in range(B)], dtype=np.float64
    )
    d_vals = np.minimum(d_vals, CLAMP)
    loss = -np.log(d_vals.mean() / float(C))
    return np.float32(loss)
